# revision 4
# baseline (speedup 1.0000x reference)
"""CrossMerge kernel for trn2 — v2 (DMA-roofline oriented).

Math (per batch element):
    means_i = mean over C of g_i              (4, H, W)
    logits  = w_proj @ means + b_proj         (4, H, W)
    w       = softmax(logits, axis=0)         (4, H, W)
    out     = sum_i g_i * w_i                 (C, H, W)

Sharding: data-parallel over batch B=8 across 8 cores; weights replicated;
no cross-device communication.

v1 was Tensor-engine bound (42 matmuls per 1024-col tile, HAM oscillating
at K=4/8 for most of each iteration -> 226us vs the 132us HBM floor).
v2 moves the product-accumulate off the PE and the bulk dtype to bf16:

  - loads: SWDGE (gpsimd) DMAs cast fp32->bf16 in flight; SBUF tiles halve,
    HBM-side traffic unchanged (the binding resource: 47.2MB @ ~358GB/s).
  - PE per 512-col slice: 8 logits MMs (bf16, fp32 PSUM), 1 denominator,
    4 softmax-weight broadcasts = 13 MMs (~2.8us warm) -- far under the
    5.9us slice DMA time, so PE clock state no longer matters.
  - ACT: exp(L)+bias, and the 4 PSUM->SBUF bf16 copies of the broadcast
    weights.
  - DVE: reciprocal, W=E*R, then 8 bf16 products (2x mode) + add tree;
    the final adds write fp32 directly into the store tile.
  - stores: plain fp32 on the sync/HWDGE queue so they never head-block
    the load queue.

Tolerance is rel_err < 2e-2; bf16 internals land ~1e-3.
"""

import os
import sys
from contextlib import ExitStack

import numpy as np

try:
    import concourse.bass as bass
except ImportError:  # fresh grading dir: concourse lives in the container repo
    sys.path.insert(0, "/opt/trn_rl_repo")
    import concourse.bass as bass

import concourse.tile as tile
from concourse import bacc, mybir
from concourse.bass_utils import run_bass_kernel_spmd

B, C, H, W = 8, 256, 96, 96
HW = H * W  # 9216
NCORES = 8
CPB = C // 128  # 2 partition chunks per core
DCOLS = 512  # columns per DMA tile == per compute slice (fp32 PSUM bank)
NDMA = HW // DCOLS  # 18
OCOLS = 2 * DCOLS  # output store granularity (1MB fp32)

F32 = mybir.dt.float32
BF16 = mybir.dt.bfloat16
U16 = mybir.dt.uint16
AF = mybir.ActivationFunctionType

_CACHE = {}


def build_program():
    nc = bacc.Bacc("TRN2", debug=False, num_devices=NCORES)

    gall_d = nc.dram_tensor("gall", [4, C, HW], F32, kind="ExternalInput").ap()
    # bf16 constants, one blob: 0-15 ws | 16-19 ones4x4 | 20-531 selmat
    cbu_d = nc.dram_tensor("cbu", [128, 532], U16, kind="ExternalInput").ap()
    # fp32 constants: col 0 = exp bias (rows 0-3)
    cf_d = nc.dram_tensor("cf", [128, 1], F32, kind="ExternalInput").ap()
    out = nc.dram_tensor("out", [C, HW], F32, kind="ExternalOutput").ap()

    with tile.TileContext(nc) as tc, ExitStack() as ctx:
        const = ctx.enter_context(tc.tile_pool(name="const", bufs=1))
        gin = ctx.enter_context(tc.tile_pool(name="gin", bufs=8))
        outp = ctx.enter_context(tc.tile_pool(name="outp", bufs=3))
        narrow = ctx.enter_context(tc.tile_pool(name="narrow", bufs=3))
        wbsb = ctx.enter_context(tc.tile_pool(name="wbsb", bufs=3))
        prod = ctx.enter_context(tc.tile_pool(name="prod", bufs=2))
        ps_L = ctx.enter_context(tc.tile_pool(name="psL", bufs=2, space="PSUM"))
        ps_S4 = ctx.enter_context(tc.tile_pool(name="psS4", bufs=2, space="PSUM"))
        ps_Wb = ctx.enter_context(tc.tile_pool(name="psWb", bufs=1, space="PSUM"))

        cbu = const.tile([128, 532], U16)
        nc.sync.dma_start(out=cbu[:], in_=cbu_d)
        cb = cbu.bitcast(BF16)
        ws = cb[:, 0:16]
        ones4x4 = cb[0:4, 16:20]
        selmat = cb[0:4, 20:532]
        cf = const.tile([128, 1], F32)
        nc.sync.dma_start(out=cf[:], in_=cf_d)
        bv = cf[0:4, 0:1]

        def slice_compute(d, gat, ot, oj):
            # --- softmax chain for this 512-col slice ---
            L = ps_L.tile([4, DCOLS], F32, tag="L")
            k = 0
            for i in range(4):
                for c in range(CPB):
                    nc.tensor.matmul(
                        L,
                        lhsT=ws[:, 4 * i : 4 * i + 4],
                        rhs=gat[:, i, c, :],
                        start=(k == 0),
                        stop=(k == 7),
                    )
                    k += 1
            E = narrow.tile([4, DCOLS], BF16, tag="E")
            nc.scalar.activation(E[:], L, AF.Exp, bias=bv, scale=1.0)
            S4 = ps_S4.tile([4, DCOLS], F32, tag="S4")
            nc.tensor.matmul(S4[:], lhsT=ones4x4, rhs=E[:], start=True, stop=True)
            # reciprocal DVE op requires base partition 0 (HW-verified in v1)
            R4 = narrow.tile([4, DCOLS], F32, tag="R4")
            nc.vector.reciprocal_approx_fast(R4[:], S4[:])
            W4 = narrow.tile([4, DCOLS], BF16, tag="W4")
            nc.gpsimd.tensor_mul(W4[:], E[:], R4[:])
            # --- broadcast weights to 128 partitions (PE) + bf16 copies ---
            wbs = []
            for i in range(4):
                Wbp = ps_Wb.tile([128, DCOLS], F32, tag=f"wb{i}")
                nc.tensor.matmul(
                    Wbp[:],
                    lhsT=selmat[:, 128 * i : 128 * (i + 1)],
                    rhs=W4[:],
                    start=True,
                    stop=True,
                )
                Wb = wbsb.tile([128, 1, DCOLS], BF16, tag=f"ws{i}")
                nc.scalar.copy(Wb[:, 0, :], Wbp[:])
                wbs.append(Wb)
            # --- products + accumulation tree, fused over the 2 c-chunks ---
            # [128, 2, 512] ops; the weight AP broadcasts over c (0-stride).
            def bmul(eng, q_ap, i):
                a, b = bass.broadcast_tensor_aps(gat[:, i], wbs[i][:])
                eng.tensor_mul(q_ap, a, b)

            q0 = prod.tile([128, CPB, DCOLS], BF16, tag="q0")
            bmul(nc.vector, q0[:], 0)
            q1 = prod.tile([128, CPB, DCOLS], BF16, tag="q1")
            bmul(nc.vector, q1[:], 1)
            s01 = prod.tile([128, CPB, DCOLS], BF16, tag="s01")
            nc.vector.tensor_add(s01[:], q0[:], q1[:])
            q2 = prod.tile([128, CPB, DCOLS], BF16, tag="q2")
            bmul(nc.vector, q2[:], 2)
            q3 = prod.tile([128, CPB, DCOLS], BF16, tag="q3")
            bmul(nc.gpsimd, q3[:], 3)
            s23 = prod.tile([128, CPB, DCOLS], BF16, tag="s23")
            nc.vector.tensor_add(s23[:], q2[:], q3[:])
            nc.vector.tensor_add(
                ot[:, :, oj * DCOLS : (oj + 1) * DCOLS], s01[:], s23[:]
            )

        ot = None
        for d in range(NDMA):
            n0 = d * DCOLS
            gat = gin.tile([128, 4, CPB, DCOLS], BF16, tag="gall")
            # SWDGE cast-load: HBM fp32 -> SBUF bf16
            nc.gpsimd.dma_start(
                out=gat[:],
                in_=gall_d[:, :, n0 : n0 + DCOLS].rearrange(
                    "i (c p) n -> p i c n", c=CPB
                ),
            )
            if d % 2 == 0:
                ot = outp.tile([128, CPB, OCOLS], F32, tag="ot")
            slice_compute(d, gat, ot, d % 2)
            if d % 2 == 1:
                N0 = (d - 1) * DCOLS
                nc.sync.dma_start(
                    out=out[:, N0 : N0 + OCOLS].rearrange(
                        "(c p) n -> p c n", c=CPB
                    ),
                    in_=ot[:],
                )

    nc.compile()
    return nc


def _get_program():
    if "nc" not in _CACHE:
        _CACHE["nc"] = build_program()
    return _CACHE["nc"]


def _to_bf16_bits(x):
    """Round-to-nearest-even fp32 -> bf16 bit pattern (uint16)."""
    u = np.asarray(x, dtype=np.float32).view(np.uint32)
    rounded = u + 0x7FFF + ((u >> 16) & 1)
    return (rounded >> 16).astype(np.uint16)


def make_consts(w_proj, b_proj):
    w = np.asarray(w_proj, dtype=np.float32)
    b = np.asarray(b_proj, dtype=np.float32)
    ws = np.empty((128, 16), dtype=np.float32)
    for i in range(4):
        for o in range(4):
            ws[:, 4 * i + o] = w[o, i] / C
    cbu = np.zeros((128, 532), dtype=np.float32)
    cbu[:, 0:16] = ws
    cbu[0:4, 16:20] = 1.0
    cbu[0:4, 20:532] = np.repeat(np.eye(4, dtype=np.float32), 128, axis=1)
    cf = np.zeros((128, 1), dtype=np.float32)
    cf[0:4, 0] = b
    return _to_bf16_bits(cbu), cf


LAST_RESULT = None


def kernel(g0, g1, g2, g3, w_proj, b_proj):
    global LAST_RESULT
    nc = _get_program()

    cbu, cf = make_consts(w_proj, b_proj)

    gall = np.stack(
        [np.asarray(x, dtype=np.float32).reshape(B, C, HW) for x in (g0, g1, g2, g3)],
        axis=1,
    )  # (B, 4, C, HW)
    in_maps = []
    for bi in range(NCORES):
        m = {"gall": np.ascontiguousarray(gall[bi]), "cbu": cbu, "cf": cf}
        in_maps.append(m)

    res = run_bass_kernel_spmd(
        nc,
        in_maps,
        list(range(NCORES)),
        trace=bool(int(os.environ.get("CM_TRACE", "0"))),
        tmpdir=os.environ.get("CM_TRACE_DIR") or None,
    )
    LAST_RESULT = res
    out_full = np.stack(
        [res.results[bi]["out"].reshape(C, H, W) for bi in range(NCORES)], axis=0
    )
    return out_full
